# revision 42
# baseline (speedup 1.0000x reference)
"""Trainium2 Bass kernel for nn_EnhancedHamiltonianEvolution.

Math: the reference's FFT -> gate -> IFFT along T is, by linearity, an exact
per-channel scaling (the gate is constant along the frequency axis, shape
[1,1,1,qd]).  The two Hamilton products with fixed (normalized) quaternions are
a per-channel linear map on the 4 components.  So the whole module is

    out[b,t,:,d] = M_d @ x[b,t,:,d],      M_d = L(ql_d) @ R(qr_conj_d) * gate_d

a pointwise 4x4 mix over qd=512 channels -- memory bound.

Kernel strategy (8 cores, data-parallel over the B*T=16384 rows):
  * Host transposes each core's row-slice to feature-major [2048, 2048] so
    device DMAs are contiguous with features on SBUF partitions.
  * Features f = j*512 + g*32 + dd are regrouped per 32-channel group g so one
    SBUF tile [128, rows] holds all 4 components j of 32 channels.  The 4x4
    mix for those channels is ONE 128x128 block-diagonal fp32 matmul on PE
    (full fp32 precision; each input element is read exactly once).
  * PSUM -> SBUF copies alternate Scalar/Vector engines; DMAs use HWDGE.
  * All HBM streams (x in, weights, y out) are fp16: the kernel is at the
    HBM-bandwidth roofline, so halving bytes halves runtime.  PE multiplies
    fp16 and accumulates fp32 in PSUM; the copy back to SBUF rounds to fp16.
    End-to-end error ~1e-3, far inside the 2e-2 gate.
"""

import sys
import types

import numpy as np

N_CORES = 8
B, T, D = 4, 4096, 2048
QD = D // 4                      # 512 channels
ROWS = B * T                     # 16384
ROWS_PER_CORE = ROWS // N_CORES  # 2048
N_GROUPS = QD // 32              # 16 groups of 32 channels
GROUPS_PER_TILE = 4              # groups fetched per DMA (slab = 2MB fp16)
N_TILE = 512                     # matmul moving free dim (one PSUM bank)

TRACE = False       # set True (by test.py) to capture an NTFF profile
LAST_RESULT = None  # BassKernelResults of the most recent kernel() call

_COMPILED = {}
_WALRUS_EXTRA_FLAGS: list = []


def _install_ntff_hook_shim():
    """bass_utils wants antenv.axon_hooks for trace=True under axon; the image
    ships only a stub antenv.  Recreate the module with the ctypes driver."""
    if "antenv.axon_hooks" in sys.modules:
        return
    from trn_agent_boot.trn_boot import _ntff_profile_via_ctypes

    hook = _ntff_profile_via_ctypes("/opt/axon/libaxon_pjrt.so")
    mod = types.ModuleType("antenv.axon_hooks")
    mod.get_axon_ntff_profile_hook = lambda: hook
    mod.set_axon_ntff_profile_hook = lambda h: None
    sys.modules["antenv.axon_hooks"] = mod
    import antenv

    antenv.axon_hooks = mod


def _build_M(q_left, q_right, spectral_gate):
    """Combined per-channel 4x4 matrix, float64 -> [4,4,QD]."""
    ql = q_left.astype(np.float64)
    qr = q_right.astype(np.float64)
    g = spectral_gate.astype(np.float64).reshape(-1)
    eps = 1e-8
    ql = ql / np.sqrt((ql * ql).sum(0, keepdims=True) + eps)
    qr = qr / np.sqrt((qr * qr).sum(0, keepdims=True) + eps)
    qc = qr * np.array([1.0, -1.0, -1.0, -1.0]).reshape(4, 1)
    w1, x1, y1, z1 = ql
    w2, x2, y2, z2 = qc
    A = np.array([[w1, -x1, -y1, -z1],
                  [x1, w1, -z1, y1],
                  [y1, z1, w1, -x1],
                  [z1, -y1, x1, w1]])
    Bm = np.array([[w2, -x2, -y2, -z2],
                   [x2, w2, z2, -y2],
                   [y2, -z2, w2, x2],
                   [z2, y2, -x2, w2]])
    return np.einsum("ikd,kjd->ijd", A, Bm) * g[None, None, :]


def _build_wdense(M):
    """Dense weight payload [128, 96] fp16: cols 0:64 hold the 4x4 channel
    mixes A[j*32+dd, g*4+i] = M[i, j, g*32+dd]; cols 64:96 hold the diagonal
    mask E[p, c] = (c == p%32).

    The device expands this to the block-diagonal PE lhsT
    W[j*32+dd, g*128 + i*32 + dd'] = M[i, j, g*32+dd] * (dd' == dd)
    with one DVE broadcast-multiply -- shipping 24KB instead of the 512KB
    mostly-zero block-diagonal matrix."""
    A = np.ascontiguousarray(
        M.reshape(4, 4, N_GROUPS, 32).transpose(1, 3, 2, 0).reshape(128, 64)
    )
    E = np.tile(np.eye(32), (4, 1))
    return np.concatenate([A, E], axis=1).astype(np.float16)


def _install_walrus_flag_patch():
    """Append --max-sem-num to the walrus invocation: its codegen emits a
    serial per-semaphore clear epilogue over ALL sems up to the cap (249
    EVENT_SEMAPHOREs ~= 6us of measured kernel time).  Bass references sems
    only up to ~173, so a lower cap keeps correctness while shrinking the
    teardown chain."""
    from concourse import bass_utils

    if getattr(bass_utils, "_walrus_flag_patch", False):
        return
    orig = bass_utils.run_command

    def patched(argv, **kwargs):
        if argv and "walrus_driver" in str(argv[0]):
            argv = list(argv) + list(_WALRUS_EXTRA_FLAGS)
        return orig(argv, **kwargs)

    bass_utils.run_command = patched
    bass_utils._walrus_flag_patch = True


def _build_nc():
    import concourse.bacc as bacc
    import concourse.mybir as mybir
    from concourse.tile import TileContext

    _install_walrus_flag_patch()

    from concourse.bass import AP

    f32 = mybir.dt.float32
    f16 = mybir.dt.float16
    nc = bacc.Bacc("TRN2", target_bir_lowering=False)
    # host pre-groups features as (g, j, dd): xt[g*128 + j*32 + dd, r]
    xt = nc.dram_tensor("xt", [D, ROWS_PER_CORE], f16, kind="ExternalInput")
    wm = nc.dram_tensor("wm", [128, 96], f16, kind="ExternalInput")
    yt = nc.dram_tensor("yt", [D, ROWS_PER_CORE], f16, kind="ExternalOutput")

    # partition-first views: [p, g, r]
    xt3 = xt.rearrange("(g p) r -> p g r", g=N_GROUPS)
    yt3 = yt.rearrange("(g p) r -> p g r", g=N_GROUPS)

    GPT = GROUPS_PER_TILE
    n_slabs = N_GROUPS // GPT
    ntiles = ROWS_PER_CORE // N_TILE

    with TileContext(nc) as tc:
        with (
            tc.tile_pool(name="w", bufs=1) as wpool,
            tc.tile_pool(name="wd", bufs=1) as wdpool,
            tc.tile_pool(name="warm", bufs=1) as warmpool,
            tc.tile_pool(name="xin", bufs=n_slabs) as xpool,
            tc.tile_pool(name="yout", bufs=n_slabs) as ypool,
            tc.tile_pool(name="ps", bufs=4, space="PSUM") as pspool,
        ):
            # PE HAM warmup: the clock-gate holds the PE at 1.2 GHz until
            # ~3.4us of sustained activity.  A few dummy matmuls on an
            # uninitialized tile (results never read) during the DMA ramp
            # flip it to 2.4 GHz before the first real matmul.
            warm = warmpool.tile([128, N_TILE], f16)
            nc.gpsimd.memset(warm, 0.0)
            # shares the "ps" slot rotation (a separate tag would need more
            # PSUM banks than exist)
            ps_dummy = pspool.tile([128, 2 * N_TILE], f32, name="ps")
            for _ in range(5):
                nc.tensor.matmul(
                    ps_dummy[:, :N_TILE], warm[:, :128], warm,
                    start=True, stop=True,
                )

            # dense weight payload (24KB) on the ACT HWDGE ring, ahead of
            # that ring's input slabs
            wd = wdpool.tile([128, 96], f16)
            nc.scalar.dma_start(out=wd, in_=wm[:, :])

            # Input rides the two HWDGE rings (SP, ACT) while output rides
            # the SWDGE queue: SDMA round-robins between queues with work
            # at packet granularity, so input gets ~2/3 of fabric bandwidth.
            # Each ring is FIFO, so each slab is SPLIT across the rings
            # (sync: first half of its groups, scalar: second half) — both
            # rings then deliver data in consumption order instead of one
            # ring prefetching slabs the pipeline won't touch for 20us.
            def issue_in(s):
                xin = xpool.tile([128, GPT * ROWS_PER_CORE], f16)
                xing = xin.rearrange("p (g r) -> p g r", g=GPT)
                half = GPT // 2
                if s == 0:
                    # slab 0's first half rides the SCALAR ring (it exits
                    # the runtime wrapper ~1-3us before sync), per-group
                    # with uniform 4KB-per-partition descriptors: group 0
                    # gates the whole compute/output chain
                    for g2 in range(half):
                        nc.scalar.dma_start(
                            out=xing[:, g2:g2 + 1], in_=xt3[:, g2:g2 + 1]
                        )
                    nc.sync.dma_start(
                        out=xing[:, half:],
                        in_=xt3[:, half:GPT],
                    )
                    return xin
                nc.sync.dma_start(
                    out=xing[:, :half],
                    in_=xt3[:, s * GPT:s * GPT + half],
                )
                nc.scalar.dma_start(
                    out=xing[:, half:],
                    in_=xt3[:, s * GPT + half:(s + 1) * GPT],
                )
                return xin

            xins = [issue_in(s) for s in range(n_slabs)]

            # Expand the dense payload to the block-diagonal PE lhsT
            #   wtile[p, g*128 + i*32 + c] = A[p, g*4+i] * E[p, c]
            # (E[p, c] = delta(c == p%32)) with DVE broadcast-multiplies:
            # in0 repeats each A column 32x (stride-0 inner dim), in1 tiles
            # E's 32 columns across the free dim (stride-0 outer dim).
            # Group 0's 128 columns go first so matmul 0 isn't gated on the
            # full expansion.
            wtile = wpool.tile([128, N_GROUPS * 128], f16)
            ebase = wd[:, 64:96]

            def expand(lo, hi):
                ncol = (hi - lo) * 4
                out_ap = wtile[:, lo * 128:hi * 128]
                out_ap = AP(
                    out_ap.tensor, out_ap.offset,
                    [out_ap.ap[0], [32, ncol], [1, 32]],
                )
                a_ap = wd[:, lo * 4:lo * 4 + ncol]
                a_ap = AP(a_ap.tensor, a_ap.offset,
                          [a_ap.ap[0], [1, ncol], [0, 32]])
                e_ap = AP(ebase.tensor, ebase.offset,
                          [ebase.ap[0], [0, ncol], [1, 32]])
                nc.vector.scalar_tensor_tensor(
                    out=out_ap, in0=a_ap, scalar=1.0, in1=e_ap,
                    op0=mybir.AluOpType.mult, op1=mybir.AluOpType.mult,
                )

            # groups 8-15 are expanded later (after group 0's copies) so
            # the first casts don't queue behind a 2us DVE op; their
            # weights are still ready ~8us before slab 2 consumes them
            expand(0, 1)
            expand(1, 8)

            copy_tick = 0
            for s in range(n_slabs):
                xin = xins[s]
                yout = ypool.tile([128, GPT * ROWS_PER_CORE], f16)
                for g2 in range(GPT):
                    g = s * GPT + g2
                    lhsT = wtile[:, g * 128:(g + 1) * 128]
                    last_group = (s == n_slabs - 1 and g2 == GPT - 1)
                    for ht in range(ntiles // 2):
                        # two matmuls fill a 2-bank PSUM tile; ONE copy
                        # drains both — halves the copy instruction count
                        # and its fixed overhead
                        ps = pspool.tile([128, 2 * N_TILE], f32)
                        base = g2 * ROWS_PER_CORE + ht * 2 * N_TILE
                        for k in range(2):
                            nc.tensor.matmul(
                                ps[:, k * N_TILE:(k + 1) * N_TILE], lhsT,
                                xin[:, base + k * N_TILE:
                                        base + (k + 1) * N_TILE],
                                start=True, stop=True,
                            )
                        sl = slice(base, base + 2 * N_TILE)
                        # group 0's copies both ride Scalar so the first
                        # out-DMA is independent of the vector queue
                        if s == 0 and g2 == 0:
                            nc.scalar.copy(yout[:, sl], ps)
                        elif copy_tick % 2 == 0:
                            nc.scalar.copy(yout[:, sl], ps)
                        else:
                            nc.vector.tensor_copy(out=yout[:, sl], in_=ps)
                        copy_tick += 1
                        # The LAST slab's outs ride the sync HWDGE ring
                        # (idle by then) per-copy: the tail drains from
                        # two queues and the final receipt is a ~0.6us
                        # HWDGE one, not ~2us SWDGE.
                        if s == n_slabs - 1:
                            nc.sync.dma_start(
                                out=yt3[:, g, ht * 2 * N_TILE:
                                              (ht + 1) * 2 * N_TILE],
                                in_=yout[:, sl],
                            )
                    # Bulk out-DMAs ride the SWDGE (gpsimd) queue per
                    # GROUP (512KB): one queue for output vs two HWDGE
                    # queues for input biases the SDMA round-robin toward
                    # input, and the ~0.7us SWDGE emission per issue
                    # sustains ~730 GB/s at this granularity (256KB
                    # issues cap the backlog drain at ~370 GB/s).
                    if s < n_slabs - 1:
                        nc.gpsimd.dma_start(
                            out=yt3[:, g],
                            in_=yout[:, g2 * ROWS_PER_CORE:
                                       (g2 + 1) * ROWS_PER_CORE],
                        )
                    if s == 0 and g2 == 0:
                        expand(8, N_GROUPS)
    nc.finalize()

    # Bass's entry block re-barriers all engines (the runtime wrapper they
    # just left already ends in a barrier) and memsets four const-AP tiles
    # this kernel never reads — ~1us of measured time.  Strip both; the
    # branch into the tile block and every cross-engine dependency there
    # is semaphore-tracked explicitly.
    main_blk = next(
        b for b in nc.m.functions[0].blocks if b.name == "main"
    )
    keep = [
        i for i in main_blk.instructions
        if type(i).__name__ not in
        ("InstMemset", "InstDrain", "InstEventSemaphore")
    ]
    del main_blk.instructions[:]
    main_blk.instructions.extend(keep)

    # The act-table load (1.28us, needed by the ACTIVATE copies) gets
    # hoisted to the head of the Activation engine's stream, delaying that
    # ring's input DMA issues.  Move it after the upfront DMA issues, just
    # before the first ACTIVATE — the ring then starts streaming at once
    # and the load still dominates every activation.
    for blk in nc.m.functions[0].blocks:
        insts = blk.instructions
        li = [i for i, x in enumerate(insts)
              if isinstance(x, mybir.InstLoadActFuncSet)]
        ai = [i for i, x in enumerate(insts)
              if isinstance(x, mybir.InstActivation)]
        if li and ai and li[0] < ai[0]:
            load = insts.pop(li[0])
            insts.insert(ai[0] - 1, load)
    return nc


def _get_nc():
    if "nc" not in _COMPILED:
        _COMPILED["nc"] = _build_nc()
    return _COMPILED["nc"]


def _run_preplaced(nc, in_maps, n_cores, trace=False):
    """Like bass2jax.run_bass_via_pjrt, but device_put + block all shards
    BEFORE dispatch.  The stock path streams H2D transfers while early cores
    already execute, so a core whose HBM-stack sibling is still uploading
    loses ~15% bandwidth (observed: even cores ~110us, odd ~95us).  With
    pre-placement every core starts with a quiet stack."""
    import jax
    from jax.experimental.shard_map import shard_map
    from jax.sharding import Mesh, NamedSharding, PartitionSpec
    import concourse.mybir as mybir
    from concourse import bass2jax

    bass2jax.install_neuronx_cc_hook()

    partition_name = (
        nc.partition_id_tensor.name if nc.partition_id_tensor else None
    )
    in_names, out_names, out_avals, zero_shapes = [], [], [], []
    for alloc in nc.m.functions[0].allocations:
        if not isinstance(alloc, mybir.MemoryLocationSet):
            continue
        name = alloc.memorylocations[0].name
        if alloc.kind == "ExternalInput":
            if name != partition_name:
                in_names.append(name)
        elif alloc.kind == "ExternalOutput":
            out_names.append(name)
            out_avals.append(
                jax.core.ShapedArray(
                    tuple(alloc.tensor_shape), mybir.dt.np(alloc.dtype)
                )
            )
            zero_shapes.append(
                (tuple(alloc.tensor_shape), mybir.dt.np(alloc.dtype))
            )
    n_params = len(in_names)
    n_outs = len(out_names)
    bind_in_names = list(in_names) + list(out_names)
    if partition_name is not None:
        bind_in_names.append(partition_name)

    def _body(*args):
        operands = list(args)
        if partition_name is not None:
            operands.append(bass2jax.partition_id_tensor())
        outs = bass2jax._bass_exec_p.bind(
            *operands,
            out_avals=tuple(out_avals),
            in_names=tuple(bind_in_names),
            out_names=tuple(out_names),
            lowering_input_output_aliases=(),
            sim_require_finite=True,
            sim_require_nnan=True,
            nc=nc,
        )
        return tuple(outs)

    devices = jax.devices()[:n_cores]
    mesh = Mesh(np.asarray(devices), ("core",))
    in_specs = (PartitionSpec("core"),) * (n_params + n_outs)
    out_specs = (PartitionSpec("core"),) * n_outs
    sharded = jax.jit(
        shard_map(
            _body, mesh=mesh, in_specs=in_specs, out_specs=out_specs,
            check_rep=False,
        ),
        donate_argnums=tuple(range(n_params, n_params + n_outs)),
        keep_unused=True,
    )
    concat_in = [
        np.concatenate(
            [np.asarray(in_maps[c][nm]) for c in range(n_cores)], axis=0
        )
        for nm in in_names
    ]
    concat_zeros = [
        np.zeros((n_cores * shp[0], *shp[1:]), dt)
        for shp, dt in zero_shapes
    ]
    shd = NamedSharding(mesh, PartitionSpec("core"))
    placed = [jax.device_put(a, shd) for a in concat_in + concat_zeros]
    placed = jax.block_until_ready(placed)

    perf = None
    if trace:
        import glob as _glob
        import tempfile
        from antenv.axon_hooks import get_axon_ntff_profile_hook
        from concourse import bass_utils
        from concourse._compat import FishPath
        from concourse.env import env_bass_perfetto_profile_all_cores
        import gauge.profiler

        hook = get_axon_ntff_profile_hook()
        tmpdir = tempfile.mkdtemp()
        trace_idx = (
            list(range(n_cores))
            if env_bass_perfetto_profile_all_cores() else [0]
        )
        with hook(tmpdir, trace_idx):
            out_arrs = jax.block_until_ready(sharded(*placed))
        if _glob.glob(tmpdir + "/*_body*.ntff"):
            sharepath = bass_utils.upload_artifacts(tmpdir)
            profile = gauge.profiler.Profile(
                profile_path=FishPath(tmpdir), kernel_dev_mode=True,
                profile_on_exit=False, bass_kernel=nc.m,
                offline_processing=True, fname="*_body*",
                metadata={"artifacts_path": sharepath},
            )
            perf = bass_utils._process_ntff_profile(
                profile, tmpdir, nc, list(range(n_cores)), None, False, {},
                trace_events=False,
            )
    else:
        out_arrs = sharded(*placed)

    out_np = [np.asarray(a) for a in out_arrs]
    results = [
        {
            name: out_np[i].reshape(n_cores, *out_avals[i].shape)[c]
            for i, name in enumerate(out_names)
        }
        for c in range(n_cores)
    ]
    if perf is not None:
        return perf.as_bass_kernel_results(results)
    from concourse.bass_utils import BassKernelResults
    return BassKernelResults(
        results=results, instructions_and_trace=None, profile_json=None,
        exec_time_ns=None,
    )


def kernel(x, q_left, q_right, spectral_gate):
    global LAST_RESULT
    from concourse.bass_utils import run_bass_kernel_spmd

    if TRACE:
        _install_ntff_hook_shim()

    M = _build_M(np.asarray(q_left), np.asarray(q_right),
                 np.asarray(spectral_gate))
    wmat = _build_wdense(M)

    x2 = np.asarray(x, dtype=np.float32).reshape(ROWS, D).astype(np.float16)
    in_maps = []
    for c in range(N_CORES):
        sl = x2[c * ROWS_PER_CORE:(c + 1) * ROWS_PER_CORE]
        # device layout: xt[g*128 + j*32 + dd, r] = x[r, j*512 + g*32 + dd]
        xt = np.ascontiguousarray(
            sl.reshape(ROWS_PER_CORE, 4, N_GROUPS, 32).transpose(2, 1, 3, 0)
        ).reshape(D, ROWS_PER_CORE)
        in_maps.append({"xt": xt, "wm": wmat})

    nc = _get_nc()
    res = None
    for attempt in range(6):
        try:
            if attempt < 3:
                # primary: pre-placed runner — device_put + block BEFORE
                # dispatch so no core executes while its HBM-stack sibling
                # is still receiving uploads (the stock path costs a
                # degraded core ~15% bandwidth and is bimodal ~55/65us)
                res = _run_preplaced(nc, in_maps, N_CORES, trace=TRACE)
            else:
                # fallback: stock dispatch path
                res = run_bass_kernel_spmd(
                    nc, in_maps, core_ids=list(range(N_CORES)), trace=TRACE
                )
            break
        except Exception:
            # sporadic NRT_EXEC_UNIT_UNRECOVERABLE has been observed on this
            # fabric; a clean retry (fresh jit dispatch) recovers
            if attempt == 5:
                raise
            import time
            time.sleep(2.0)
    LAST_RESULT = res

    out = np.empty((ROWS, D), dtype=np.float32)
    for c in range(N_CORES):
        # yt[g*128 + i*32 + dd, r] -> out[r, i*512 + g*32 + dd]
        yt = res.results[c]["yt"].reshape(N_GROUPS, 4, 32, ROWS_PER_CORE)
        out[c * ROWS_PER_CORE:(c + 1) * ROWS_PER_CORE] = (
            yt.transpose(3, 1, 0, 2).reshape(ROWS_PER_CORE, D)
        )
    return out.reshape(B, T, D)

